# revision 16
# baseline (speedup 1.0000x reference)
"""Causal selective self-attention Trainium2 kernel (8 NeuronCores).

Sharding: core c handles batch b = c//4 and heads [3g, 3g+3) where g = c%4.
The selective-S matrix (per-batch [T,T], reduced over all 12 heads) is
computed as per-core partials over the core's own 3 heads and AllReduced
across the 4 cores of each batch.

Layouts are feature-major ("transposed"): q/k are stored [head_dim, T] so
that every matmul's stationary (lhsT) and moving (rhs) operands come out
of the preceding GEMM directly, with no on-device transposes.

Math notes:
  - softmax is computed without max-subtraction: logits = scale*q.k - FF
    with FF >= 0 and |scale*q.k| <~ 2.5, so exp never overflows, and the
    protected BOS column (FF[:,0] == 0) lower-bounds each row's Z.
  - Everything is fp16 storage with fp32 PSUM accumulation and an fp32
    scan state for the FF cumsum (tensor_tensor_scan keeps fp32 state
    regardless of operand dtype), so the T-row accumulation does not
    amplify the fp16 rounding of individual S entries.
  - The FF subtraction happens inside the logit PSUM accumulation via an
    identity-matmul whose rhs is the (negated) FF block, so phase 2 does a
    single exp per tile with no separate exp(-FF) tensor or DVE multiply.
  - The causal/diagonal mask rides in as -60000 baked into the FF blocks.

Schedule notes:
  - S^T blocks are emitted per k n-super so AllReduce chunk 0 launches as
    early as possible; the v GEMM fills the collective window.
  - FF-scan input loads use the sync DMA queue: the gpsimd queue holds the
    collectives, and an in-order queue behind 4 ARs would stall the first
    scan until the last AR issued.
  - All startup constants arrive in 4 packed DMAs (descriptor issue is
    ~0.6us each on the DGE queue; 35 small DMAs would cost ~20us serial).
  - S partials are evacuated PSUM->SBUF on ACT (idle during the S GEMM;
    PSUM-sourced DVE ops pay a ~125ns port penalty and DVE is busier).
"""

import numpy as np

import concourse.bass as bass
import concourse.bacc as bacc
import concourse.mybir as mybir
import concourse.tile as tile
from contextlib import ExitStack
from concourse.bass_utils import run_bass_kernel_spmd

dt = mybir.dt
AF = mybir.ActivationFunctionType
ALU = mybir.AluOpType

B, T, C, H, HD = 2, 2048, 768, 12, 64
N_CORES = 8
HPC = 3                # heads per core
D = HPC * HD           # 192 feature dims per core
DV = HPC * 65          # v feature dims incl. ones column per head
NB = T // 128          # 16 query/key blocks of 128
NS = T // 512          # 4 i-supers of 512
CC = C // 128          # 6 contraction chunks
SCALE = 1.0 / np.sqrt(HD)

S_DT = dt.float16      # S partials + AllReduce dtype
NEGBIG = -60000.0      # causal mask additive (fp16-safe; exp -> 0)

# c16 packed-constant column offsets: maskM | maskA | ident | bva | wpA | wpB
C16_MASKM, C16_MASKA, C16_ID = 0, 128, 256
C16_BVA, C16_WPA = 384, 384 + DV
C16_WPB = C16_WPA + C
C16_W = C16_WPB + C

# triangular-packed S scratch: block bj holds cols i in [128*bj, T)
BLK_LEN = [T - 128 * bj for bj in range(NB)]
# 4 contiguous DRAM chunks of 4 blocks each (separate tensors => collectives
# operate on plain contiguous buffers)
CHUNK_LEN = [sum(BLK_LEN[4 * k:4 * k + 4]) for k in range(4)]
BLK_OFF = []  # (chunk, offset within chunk)
for bj in range(NB):
    k = bj // 4
    off = sum(BLK_LEN[4 * k:bj])
    BLK_OFF.append((k, off))

_NC_CACHE = {}
NO_AR = False  # ablation: replace AllReduce with local copy (wrong numerics)


def build_nc(reps=1):
    key = (reps, NO_AR)
    if key in _NC_CACHE:
        return _NC_CACHE[key]
    nc = bacc.Bacc("TRN2", target_bir_lowering=False, debug=False,
                   num_devices=N_CORES)

    # host-swizzled: one contiguous DMA each
    xT = nc.declare_dram_parameter("xT", [128, CC * T], dt.float16, isOutput=False)
    wA = nc.declare_dram_parameter("wA", [128, CC * (2 * D + DV)], dt.float16, isOutput=False)
    c16 = nc.declare_dram_parameter("c16", [128, C16_W], dt.float16, isOutput=False)
    c32 = nc.declare_dram_parameter("c32", [128, 6], dt.float32, isOutput=False)
    out = nc.declare_dram_parameter("out", [T, C], dt.float32, isOutput=True)

    ios = (xT, wA, c16, c32, out)
    with tile.TileContext(nc) as tc:
        for _rep in range(reps):
            _emit_body(nc, tc, ios)

    nc.compile()
    _NC_CACHE[key] = nc
    return nc


def _emit_body(nc, tc, ios):
    (xT, wA, c16, c32, out) = ios
    with ExitStack() as ctx:
        dram = ctx.enter_context(tc.tile_pool(name="dram", bufs=1, space="DRAM"))
        st_w = [dram.tile([128, CHUNK_LEN[k]], S_DT, name=f"stw{k}", tag=f"stw{k}") for k in range(4)]
        st_r = [dram.tile([128, CHUNK_LEN[k]], S_DT, name=f"str{k}", tag=f"str{k}") for k in range(4)]

        # ---- long-lived SBUF tensors ----
        persist = ctx.enter_context(tc.tile_pool(name="persist", bufs=1))
        # q/k feature-major fp16 (m0: dims 0..128 = heads 0,1; m1: head 2)
        qT = [persist.tile([128, T], dt.float16, name="qT0", tag="qT0"),
              persist.tile([64, T], dt.float16, name="qT1", tag="qT1")]
        kT = [persist.tile([128, T], dt.float16, name="kT0", tag="kT0"),
              persist.tile([64, T], dt.float16, name="kT1", tag="kT1")]
        # v (token-major) incl. ones col per head: block tb at cols [tb*DV, ...)
        vaug = persist.tile([128, NB * DV], dt.float16, tag="vaug")
        c16_t = persist.tile([128, C16_W], dt.float16, tag="c16")
        c32_t = persist.tile([128, 6], dt.float32, tag="c32")
        zeros_t = persist.tile([128, T], S_DT, tag="zeros")
        # -FF^T per j-block, fp16, lives through phase 2 (4.5 MB)
        ffst = [persist.tile([128, BLK_LEN[bj]], dt.float16, name=f"ffst{bj}", tag=f"ffst{bj}")
                for bj in range(NB)]

        nc.sync.dma_start(c16_t[:], c16[:])
        nc.sync.dma_start(c32_t[:], c32[:])
        nc.vector.memset(zeros_t[:], 0.0)
        maskm_t = c16_t[:, C16_MASKM:C16_MASKM + 128]
        maska_t = c16_t[:, C16_MASKA:C16_MASKA + 128]
        ident_t = c16_t[:, C16_ID:C16_ID + 128]
        bva_t = c16_t[:, C16_BVA:C16_BVA + DV]
        wp_t = [c16_t[:, C16_WPA:C16_WPA + C], c16_t[0:64, C16_WPB:C16_WPB + C]]
        bq_t, bk_t, selv_t = c32_t[:, 0:2], c32_t[:, 2:4], c32_t[:, 4:6]

        MS = [(0, 128), (128, 64)]  # (dim offset, size) of the two m-tiles

        # phase-2 SBUF pools live at ctx level: if they shared addresses with
        # x16/w they would inherit a wait on the v GEMM (last x16 reader) and
        # stall the s=0 attention chain behind it.
        p1sta = ctx.enter_context(tc.tile_pool(name="p1sta", bufs=2))
        p2sb = ctx.enter_context(tc.tile_pool(name="p2sb", bufs=4))
        p2y = ctx.enter_context(tc.tile_pool(name="p2y", bufs=2))
        p2o = ctx.enter_context(tc.tile_pool(name="p2o", bufs=2))

        # ================= phase 0/1: qkv GEMMs + S partials =================
        with tc.tile_pool(name="p0", bufs=1) as p0, \
             tc.tile_pool(name="p0psum", bufs=2, space="PSUM") as p0ps, \
             tc.tile_pool(name="p1ps", bufs=4, space="PSUM") as p1ps, \
             tc.tile_pool(name="vps", bufs=2, space="PSUM") as vps, \
             tc.tile_pool(name="p1st", bufs=3) as p1st:
            x16 = p0.tile([128, CC * T], dt.float16, tag="x16")
            w_t = p0.tile([128, CC * (2 * D + DV)], dt.float16, tag="w")
            nc.sync.dma_start(x16[:], xT[:])
            nc.sync.dma_start(w_t[:], wA[:])
            wq_t = w_t[:, 0:CC * D]
            wk_t = w_t[:, CC * D:2 * CC * D]
            wv_t = w_t[:, 2 * CC * D:]

            # q GEMM (all n-supers), then qsT = q * sel_w
            qsT = [p0.tile([128, T], dt.float16, name="qsT0", tag="qsT0"),
                   p0.tile([64, T], dt.float16, name="qsT1", tag="qsT1")]
            for mi, (mof, msz) in enumerate(MS):
                for n in range(4):
                    ps = p0ps.tile([128, 512], dt.float32, tag="qk_ps")
                    for c in range(CC):
                        nc.tensor.matmul(
                            ps[:msz, :], wq_t[:, c * D + mof: c * D + mof + msz],
                            x16[:, c * T + n * 512: c * T + (n + 1) * 512],
                            start=(c == 0), stop=(c == CC - 1))
                    nc.scalar.activation(qT[mi][:, n * 512:(n + 1) * 512],
                                         ps[:msz, :], AF.Identity,
                                         bias=bq_t[:msz, mi:mi + 1])
                nc.vector.tensor_scalar_mul(qsT[mi][:], qT[mi][:],
                                            selv_t[:msz, mi:mi + 1])

            # k GEMM per n-super, then S^T blocks 4n..4n+3 + AllReduce chunk n
            # (S partials launch their collective as early as possible; the
            # v GEMM afterwards fills the collective window with PE work)
            for n in range(4):
                for mi, (mof, msz) in enumerate(MS):
                    ps = p0ps.tile([128, 512], dt.float32, tag="qk_ps")
                    for c in range(CC):
                        nc.tensor.matmul(
                            ps[:msz, :], wk_t[:, c * D + mof: c * D + mof + msz],
                            x16[:, c * T + n * 512: c * T + (n + 1) * 512],
                            start=(c == 0), stop=(c == CC - 1))
                    nc.scalar.activation(kT[mi][:, n * 512:(n + 1) * 512],
                                         ps[:msz, :], AF.Identity,
                                         bias=bk_t[:msz, mi:mi + 1])
                for bj in range(4 * n, 4 * n + 4):
                    L = BLK_LEN[bj]
                    chunk, col0 = BLK_OFF[bj]
                    sblk = p1st.tile([128, T], S_DT, tag="sblk")
                    for nn in range((L + 511) // 512):
                        nsz = min(512, L - nn * 512)
                        ps = p1ps.tile([128, 512], dt.float32, tag="s_ps")
                        i0 = bj * 128 + nn * 512
                        nc.tensor.matmul(ps[:, :nsz], kT[0][:, bj * 128:(bj + 1) * 128],
                                         qsT[0][:, i0:i0 + nsz], start=True, stop=False)
                        nc.tensor.matmul(ps[:, :nsz], kT[1][:, bj * 128:(bj + 1) * 128],
                                         qsT[1][:, i0:i0 + nsz], start=False, stop=True)
                        # alternate the PSUM evacuation between DVE and ACT so
                        # neither engine serializes the S -> AllReduce path
                        if (bj + nn) % 2 == 0:
                            nc.vector.tensor_copy(sblk[:, nn * 512:nn * 512 + nsz],
                                                  ps[:, :nsz])
                        else:
                            nc.scalar.activation(sblk[:, nn * 512:nn * 512 + nsz],
                                                 ps[:, :nsz], AF.Copy)
                    nc.sync.dma_start(st_w[n][:, col0:col0 + L], sblk[:, :L])
                if NO_AR:
                    nc.gpsimd.dma_start(st_r[n][:], st_w[n][:])
                else:
                    nc.gpsimd.collective_compute(
                        "AllReduce", ALU.add,
                        replica_groups=[[0, 1, 2, 3], [4, 5, 6, 7]],
                        ins=[st_w[n][:]], outs=[st_r[n][:]])

            # v: fp16 GEMM, token-major, 65-wide per-head slots (ones column
            # comes from a zero weight column + the +1 in bva)
            for tb in range(NB):
                ps = vps.tile([128, DV], dt.float32, tag="v_ps")
                for c in range(CC):
                    nc.tensor.matmul(
                        ps[:], x16[:, c * T + tb * 128: c * T + (tb + 1) * 128],
                        wv_t[:, c * DV:(c + 1) * DV],
                        start=(c == 0), stop=(c == CC - 1))
                nc.vector.tensor_add(vaug[:, tb * DV:(tb + 1) * DV], ps[:], bva_t[:])

        # ============ phase 1b + 2, interleaved per i-super ==================
        # PSUM pool order matters: ptp reuses the banks freed earliest
        # (qk/s GEMMs), pjp reuses the v banks (freed last, needed last).
        with tc.tile_pool(name="p2pt", bufs=2, space="PSUM") as ptp, \
             tc.tile_pool(name="p2yt", bufs=2, space="PSUM") as ytp, \
             tc.tile_pool(name="p2pj", bufs=1, space="PSUM") as pjp:
            for s in range(NS):
                # FF^T scan for AR chunk s (blocks 4s..4s+3): i-super s only
                # needs ffst[0..4s+3], so attention for super s starts after
                # chunk s arrives while later chunks are still reducing.
                for bj in range(4 * s, 4 * s + 4):
                    L = BLK_LEN[bj]
                    chunk, col0 = BLK_OFF[bj]
                    sta = p1sta.tile([128, T], S_DT, tag="sta")
                    nc.sync.dma_start(sta[:, :L], st_r[chunk][:, col0:col0 + L])
                    # relu + diag mask on gpsimd: it idles while DVE carries
                    # the scans and the phase-2 normalize chain
                    nc.gpsimd.tensor_scalar_max(sta[:, :L], sta[:, :L], 0.0)
                    if bj == 0:
                        nc.gpsimd.memset(sta[0:1, :L], 0.0)
                    nc.gpsimd.tensor_mul(sta[:, 0:128], sta[:, 0:128], maskm_t)
                    ff = ffst[bj]
                    nc.vector.memset(ff[:, 0:1], 0.0)
                    if L > 1:
                        # state -= S  => ff holds -cumsum(S) (exclusive)
                        nc.vector.tensor_tensor_scan(
                            ff[:, 1:L], zeros_t[:, 0:L - 1], sta[:, 0:L - 1],
                            0.0, ALU.add, ALU.subtract)
                    # causal/diagonal mask: -60000 where i < j
                    nc.vector.tensor_add(ff[:, 0:128], ff[:, 0:128], maska_t)
                yt_sb = [p2y.tile([128, 512], dt.float16, name="ytA", tag="ytA"),
                         p2y.tile([64, 512], dt.float16, name="ytB", tag="ytB")]
                for h in range(HPC):
                    # head h dims live at rows [h*64, h*64+64) of the m-tiles
                    (qsrc, qof) = (0, h * 64) if h < 2 else (1, 0)
                    yt_ps = ytp.tile([65, 512], dt.float32, tag="yt_ps")
                    # j-blocks in pairs sharing one 2-bank PSUM tile; the FF
                    # subtraction rides the accumulation as ident @ (-FF).
                    # attV matmuls are emitted one pair late so the PE stream
                    # never stalls on the exp of the pair it just produced.
                    pend = None
                    for pj in range(2 * s + 2):
                        pt = ptp.tile([128, 1024], dt.float32, tag="pt")
                        et = p2sb.tile([128, 1024], dt.float16, tag="et")
                        spans = []
                        for half, bj in ((0, 2 * pj), (1, 2 * pj + 1)):
                            delta = bj - 4 * s
                            ioff = 128 * delta if delta >= 0 else 0
                            npr = 512 - ioff
                            i0 = s * 512 + ioff          # global i start
                            floc = i0 - bj * 128         # col offset inside ffst[bj]
                            co = 512 * half
                            nc.tensor.matmul(pt[:, co:co + npr],
                                             kT[qsrc][qof:qof + 64, bj * 128:(bj + 1) * 128],
                                             qT[qsrc][qof:qof + 64, i0:i0 + npr],
                                             start=True, stop=False)
                            nc.tensor.matmul(pt[:, co:co + npr], ident_t,
                                             ffst[bj][:, floc:floc + npr],
                                             start=False, stop=True)
                            spans.append((bj, ioff, npr, co))
                        if spans[0][2] == 512:   # contiguous: one exp
                            w = 512 + spans[1][2]
                            nc.scalar.activation(et[:, :w], pt[:, :w], AF.Exp)
                        else:                    # gap: exp only valid halves
                            for bj, ioff, npr, co in spans:
                                nc.scalar.activation(et[:, co:co + npr],
                                                     pt[:, co:co + npr], AF.Exp)
                        if pend is not None:
                            pspans, pet = pend
                            for bj, ioff, npr, co in pspans:
                                vbase = bj * DV + h * 65
                                nc.tensor.matmul(yt_ps[:, ioff:512],
                                                 vaug[:, vbase:vbase + 65],
                                                 pet[:, co:co + npr],
                                                 start=(bj == 0), stop=False)
                        pend = (spans, et)
                    pspans, pet = pend
                    for bj, ioff, npr, co in pspans:
                        vbase = bj * DV + h * 65
                        nc.tensor.matmul(yt_ps[:, ioff:512],
                                         vaug[:, vbase:vbase + 65],
                                         pet[:, co:co + npr],
                                         start=(bj == 0), stop=(bj == 4 * s + 3))
                    # normalize: yt[d, i] * (1 / sumexp[i])
                    rs = p2sb.tile([1, 512], dt.float32, tag="rs")
                    nc.vector.reciprocal(rs[:], yt_ps[64:65, :])
                    bc_sb = p2sb.tile([64, 512], dt.float32, tag="bc_sb")
                    nc.gpsimd.partition_broadcast(bc_sb[:], rs[:])
                    (dsti, dof) = (0, h * 64) if h < 2 else (1, 0)
                    nc.vector.tensor_mul(yt_sb[dsti][dof:dof + 64, :],
                                         yt_ps[0:64, :], bc_sb[:])
                # output projection for this i-super
                for ib in range(4):
                    po = pjp.tile([128, C], dt.float32, tag="po")
                    for nof, nsz in ((0, 512), (512, 256)):
                        nc.tensor.matmul(po[:, nof:nof + nsz],
                                         yt_sb[0][:, ib * 128:(ib + 1) * 128],
                                         wp_t[0][:, nof:nof + nsz],
                                         start=True, stop=False)
                        nc.tensor.matmul(po[:, nof:nof + nsz],
                                         yt_sb[1][:, ib * 128:(ib + 1) * 128],
                                         wp_t[1][:, nof:nof + nsz],
                                         start=False, stop=True)
                    ost = p2o.tile([128, C], dt.float32, tag="ost")
                    nc.vector.tensor_copy(ost[:], po[:])
                    r0 = s * 512 + ib * 128
                    nc.sync.dma_start(out[r0:r0 + 128, :], ost[:])


def _swizzle(w, width):
    """[CC*128, width] -> [128, CC*width] with chunk c at cols [c*width, ...)."""
    return np.ascontiguousarray(
        w.reshape(CC, 128, width).transpose(1, 0, 2).reshape(128, CC * width))


def _prep_core_inputs(x, w_attn, b_attn, w_proj, b_proj, sel_w, core):
    b, g = core // 4, core % 4
    h0 = 3 * g
    rows = slice(64 * h0, 64 * (h0 + HPC))
    f32, f16 = np.float32, np.float16
    wq = (w_attn[rows, :].T * SCALE).astype(f16)                       # [768, 192]
    wk = w_attn[C + 64 * h0: C + 64 * (h0 + HPC), :].T.astype(f16)
    wv = w_attn[2 * C + 64 * h0: 2 * C + 64 * (h0 + HPC), :].T.astype(f16)
    wv_aug = np.zeros((C, DV), f16)                                    # ones-col slot
    for h in range(HPC):
        wv_aug[:, h * 65: h * 65 + 64] = wv[:, h * 64:(h + 1) * 64]
    wA = np.concatenate([_swizzle(wq, D), _swizzle(wk, D), _swizzle(wv_aug, DV)], axis=1)

    bva = np.zeros((1, DV), f32)
    for h in range(HPC):
        bva[0, h * 65: h * 65 + 64] = b_attn[2 * C + 64 * (h0 + h): 2 * C + 64 * (h0 + h + 1)]
        bva[0, h * 65 + 64] = 1.0
    c16 = np.zeros((128, C16_W), f16)
    c16[:, C16_MASKM:C16_MASKM + 128] = np.triu(np.ones((128, 128), f32), 1)
    c16[:, C16_MASKA:C16_MASKA + 128] = np.tril(np.full((128, 128), NEGBIG, f32), -1)
    c16[:, C16_ID:C16_ID + 128] = np.eye(128, dtype=f32)
    c16[:, C16_BVA:C16_BVA + DV] = np.tile(bva, (128, 1))
    c16[:, C16_WPA:C16_WPA + C] = w_proj[:, 64 * h0: 64 * h0 + 128].T.astype(f16)
    c16[0:64, C16_WPB:C16_WPB + C] = w_proj[:, 64 * h0 + 128: 64 * h0 + 192].T.astype(f16)

    c32 = np.zeros((128, 6), f32)
    c32[:, 0] = b_attn[64 * h0: 64 * h0 + 128] * np.float32(SCALE)
    c32[0:64, 1] = b_attn[64 * (h0 + 2): 64 * (h0 + 3)] * np.float32(SCALE)
    c32[:, 2] = b_attn[C + 64 * h0: C + 64 * h0 + 128]
    c32[0:64, 3] = b_attn[C + 64 * (h0 + 2): C + 64 * (h0 + 3)]
    c32[:, 4] = np.repeat(sel_w.astype(f32)[h0:h0 + 2], HD)
    c32[0:64, 5] = np.repeat(sel_w.astype(f32)[h0 + 2:h0 + 3], HD)

    return {
        "xT": _swizzle(np.ascontiguousarray(x[b].T).astype(f16), T),
        "wA": wA,
        "c16": c16,
        "c32": c32,
    }


def kernel(x, w_attn, b_attn, w_proj, b_proj, sel_w):
    x = np.asarray(x); w_attn = np.asarray(w_attn); b_attn = np.asarray(b_attn)
    w_proj = np.asarray(w_proj); b_proj = np.asarray(b_proj); sel_w = np.asarray(sel_w)
    nc = build_nc()
    in_maps = [_prep_core_inputs(x, w_attn, b_attn, w_proj, b_proj, sel_w, c)
               for c in range(N_CORES)]
    res = run_bass_kernel_spmd(nc, in_maps, list(range(N_CORES)))
    out = np.zeros((B, T, C), np.float32)
    for c in range(N_CORES):
        out[c // 4] += res.results[c]["out"]
    out += b_proj.astype(np.float32)
    return out


# revision 23
# speedup vs baseline: 3.0097x; 3.0097x over previous
"""Causal selective self-attention Trainium2 kernel (8 NeuronCores).

Sharding: core c handles batch b = c//4 and heads [3g, 3g+3) where g = c%4.
The selective-S matrix (per-batch [T,T], reduced over all 12 heads) is
computed as per-core partials over the core's own 3 heads and AllReduced
across the 4 cores of each batch.

Layouts are feature-major ("transposed"): q/k are stored [head_dim, T] so
that every matmul's stationary (lhsT) and moving (rhs) operands come out
of the preceding GEMM directly, with no on-device transposes.

Math notes:
  - softmax is computed without max-subtraction: logits = scale*q.k - FF
    with FF >= 0 and |scale*q.k| <~ 2.5, so exp never overflows, and the
    protected BOS column (FF[:,0] == 0) lower-bounds each row's Z.
  - Everything is fp16 storage with fp32 PSUM accumulation and an fp32
    scan state for the FF cumsum (tensor_tensor_scan keeps fp32 state
    regardless of operand dtype), so the T-row accumulation does not
    amplify the fp16 rounding of individual S entries.
  - The FF subtraction happens inside the logit PSUM accumulation via an
    identity-matmul whose rhs is the (negated) FF block, so phase 2 does a
    single exp per tile with no separate exp(-FF) tensor or DVE multiply.
  - The causal/diagonal mask rides in as -60000 baked into the FF blocks.

Schedule notes:
  - S^T blocks are emitted per k n-super so AllReduce chunk 0 launches as
    early as possible; the v GEMM fills the collective window.
  - FF-scan input loads use the sync DMA queue: the gpsimd queue holds the
    collectives, and an in-order queue behind 4 ARs would stall the first
    scan until the last AR issued.
  - All startup constants arrive in 4 packed DMAs (descriptor issue is
    ~0.6us each on the DGE queue; 35 small DMAs would cost ~20us serial).
  - S partials are evacuated PSUM->SBUF on ACT (idle during the S GEMM;
    PSUM-sourced DVE ops pay a ~125ns port penalty and DVE is busier).
"""

import numpy as np

import concourse.bass as bass
import concourse.bacc as bacc
import concourse.mybir as mybir
import concourse.tile as tile
from contextlib import ExitStack
from concourse.bass_utils import run_bass_kernel_spmd

dt = mybir.dt
AF = mybir.ActivationFunctionType
ALU = mybir.AluOpType

B, T, C, H, HD = 2, 2048, 768, 12, 64
N_CORES = 8
HPC = 3                # heads per core
D = HPC * HD           # 192 feature dims per core
DV = HPC * 65          # v feature dims incl. ones column per head
NB = T // 128          # 16 query/key blocks of 128
NS = T // 512          # 4 i-supers of 512
CC = C // 128          # 6 contraction chunks
SCALE = 1.0 / np.sqrt(HD)

S_DT = dt.float32      # S partials + AllReduce dtype
NEGBIG = -60000.0      # causal mask additive (fp16-safe; exp -> 0)

# c16 packed-constant column offsets: maskM | maskA | ident | bva | wpA | wpB
C16_MASKM, C16_MASKA, C16_ID = 0, 128, 256
C16_ONES = 384
C16_BVA, C16_WPA = 448, 448 + DV
C16_WPB = C16_WPA + C
C16_W = C16_WPB + C

# triangular-packed S scratch: block bj holds cols i in [128*bj, T)
BLK_LEN = [T - 128 * bj for bj in range(NB)]
# 4 contiguous DRAM chunks of 4 blocks each (separate tensors => collectives
# operate on plain contiguous buffers)
CHUNK_LEN = [sum(BLK_LEN[4 * k:4 * k + 4]) for k in range(4)]
BLK_OFF = []  # (chunk, offset within chunk)
for bj in range(NB):
    k = bj // 4
    off = sum(BLK_LEN[4 * k:bj])
    BLK_OFF.append((k, off))

_NC_CACHE = {}
NO_AR = False  # ablation: replace AllReduce with local copy (wrong numerics)


def build_nc(reps=1):
    key = (reps, NO_AR)
    if key in _NC_CACHE:
        return _NC_CACHE[key]
    nc = bacc.Bacc("TRN2", target_bir_lowering=False, debug=False,
                   num_devices=N_CORES)

    # host-swizzled: one contiguous DMA each
    xT = nc.declare_dram_parameter("xT", [128, CC * T], dt.float16, isOutput=False)
    wA = nc.declare_dram_parameter("wA", [128, CC * (2 * D + DV)], dt.float16, isOutput=False)
    c16 = nc.declare_dram_parameter("c16", [128, C16_W], dt.float16, isOutput=False)
    c32 = nc.declare_dram_parameter("c32", [128, 6], dt.float32, isOutput=False)
    out = nc.declare_dram_parameter("out", [T, C], dt.float32, isOutput=True)

    ios = (xT, wA, c16, c32, out)
    with tile.TileContext(nc) as tc:
        for _rep in range(reps):
            _emit_body(nc, tc, ios)

    nc.compile()
    _NC_CACHE[key] = nc
    return nc


def _emit_body(nc, tc, ios):
    (xT, wA, c16, c32, out) = ios
    with ExitStack() as ctx:
        dram = ctx.enter_context(tc.tile_pool(name="dram", bufs=1, space="DRAM"))
        st_w = [dram.tile([128, CHUNK_LEN[k]], S_DT, name=f"stw{k}", tag=f"stw{k}") for k in range(4)]
        st_r = [dram.tile([128, CHUNK_LEN[k]], S_DT, name=f"str{k}", tag=f"str{k}") for k in range(4)]

        # ---- long-lived SBUF tensors ----
        persist = ctx.enter_context(tc.tile_pool(name="persist", bufs=1))
        # q/k feature-major fp16 (m0: dims 0..128 = heads 0,1; m1: head 2)
        qT = [persist.tile([128, T], dt.float16, name="qT0", tag="qT0"),
              persist.tile([64, T], dt.float16, name="qT1", tag="qT1")]
        kT = [persist.tile([128, T], dt.float16, name="kT0", tag="kT0"),
              persist.tile([64, T], dt.float16, name="kT1", tag="kT1")]
        # v (token-major) incl. ones col per head: block tb at cols [tb*DV, ...)
        vaug = persist.tile([128, NB * DV], dt.float16, tag="vaug")
        c16_t = persist.tile([128, C16_W], dt.float16, tag="c16")
        c32_t = persist.tile([128, 6], dt.float32, tag="c32")
        zeros_t = persist.tile([128, T], S_DT, tag="zeros")
        # -FF^T per j-block, fp16, lives through phase 2 (4.5 MB)
        ffst = [persist.tile([128, BLK_LEN[bj]], dt.float16, name=f"ffst{bj}", tag=f"ffst{bj}")
                for bj in range(NB)]

        nc.sync.dma_start(c16_t[:], c16[:])
        nc.sync.dma_start(c32_t[:], c32[:])
        nc.vector.memset(zeros_t[:], 0.0)
        maskm_t = c16_t[:, C16_MASKM:C16_MASKM + 128]
        maska_t = c16_t[:, C16_MASKA:C16_MASKA + 128]
        ident_t = c16_t[:, C16_ID:C16_ID + 128]
        bva_t = c16_t[:, C16_BVA:C16_BVA + DV]
        wp_t = [c16_t[:, C16_WPA:C16_WPA + C], c16_t[0:64, C16_WPB:C16_WPB + C]]
        bq_t, bk_t, selv_t = c32_t[:, 0:2], c32_t[:, 2:4], c32_t[:, 4:6]

        MS = [(0, 128), (128, 64)]  # (dim offset, size) of the two m-tiles

        # phase-2 SBUF pools live at ctx level: if they shared addresses with
        # x16/w they would inherit a wait on the v GEMM (last x16 reader) and
        # stall the s=0 attention chain behind it.
        p1sta = ctx.enter_context(tc.tile_pool(name="p1sta", bufs=2))
        p2sb = ctx.enter_context(tc.tile_pool(name="p2sb", bufs=4))
        p2y = ctx.enter_context(tc.tile_pool(name="p2y", bufs=2))
        p2o = ctx.enter_context(tc.tile_pool(name="p2o", bufs=2))

        # ================= phase 0/1: qkv GEMMs + S partials =================
        with tc.tile_pool(name="p0", bufs=1) as p0, \
             tc.tile_pool(name="p0psum", bufs=2, space="PSUM") as p0ps, \
             tc.tile_pool(name="p1ps", bufs=4, space="PSUM") as p1ps, \
             tc.tile_pool(name="vps", bufs=2, space="PSUM") as vps, \
             tc.tile_pool(name="p1st", bufs=3) as p1st:
            x16 = p0.tile([128, CC * T], dt.float16, tag="x16")
            w_t = p0.tile([128, CC * (2 * D + DV)], dt.float16, tag="w")
            # chunked loads: a single huge DMA does not spray well across
            # the 16 DMA engines on HW (sim models it as parallel)
            NXC = 8
            xc = CC * T // NXC
            for i in range(NXC):
                nc.sync.dma_start(x16[:, i * xc:(i + 1) * xc], xT[:, i * xc:(i + 1) * xc])
            wc = CC * (2 * D + DV) // 6
            for i in range(6):
                nc.sync.dma_start(w_t[:, i * wc:(i + 1) * wc], wA[:, i * wc:(i + 1) * wc])
            wq_t = w_t[:, 0:CC * D]
            wk_t = w_t[:, CC * D:2 * CC * D]
            wv_t = w_t[:, 2 * CC * D:]

            # q GEMM (all n-supers), then qsT = q * sel_w
            qsT = [p0.tile([128, T], dt.float16, name="qsT0", tag="qsT0"),
                   p0.tile([64, T], dt.float16, name="qsT1", tag="qsT1")]
            for mi, (mof, msz) in enumerate(MS):
                for n in range(4):
                    ps = p0ps.tile([128, 512], dt.float32, tag="qk_ps")
                    for c in range(CC):
                        nc.tensor.matmul(
                            ps[:msz, :], wq_t[:, c * D + mof: c * D + mof + msz],
                            x16[:, c * T + n * 512: c * T + (n + 1) * 512],
                            start=(c == 0), stop=(c == CC - 1))
                    nc.scalar.activation(qT[mi][:, n * 512:(n + 1) * 512],
                                         ps[:msz, :], AF.Identity,
                                         bias=bq_t[:msz, mi:mi + 1])
                nc.vector.tensor_scalar_mul(qsT[mi][:], qT[mi][:],
                                            selv_t[:msz, mi:mi + 1])

            # k GEMM per n-super, then S^T blocks 4n..4n+3 + AllReduce chunk n
            # (S partials launch their collective as early as possible; the
            # v GEMM afterwards fills the collective window with PE work)
            for n in range(4):
                for mi, (mof, msz) in enumerate(MS):
                    ps = p0ps.tile([128, 512], dt.float32, tag="qk_ps")
                    for c in range(CC):
                        nc.tensor.matmul(
                            ps[:msz, :], wk_t[:, c * D + mof: c * D + mof + msz],
                            x16[:, c * T + n * 512: c * T + (n + 1) * 512],
                            start=(c == 0), stop=(c == CC - 1))
                    nc.scalar.activation(kT[mi][:, n * 512:(n + 1) * 512],
                                         ps[:msz, :], AF.Identity,
                                         bias=bk_t[:msz, mi:mi + 1])
                for bj in range(4 * n, 4 * n + 4):
                    L = BLK_LEN[bj]
                    chunk, col0 = BLK_OFF[bj]
                    sblk = p1st.tile([128, T], S_DT, tag="sblk")
                    for nn in range((L + 511) // 512):
                        nsz = min(512, L - nn * 512)
                        ps = p1ps.tile([128, 512], dt.float32, tag="s_ps")
                        i0 = bj * 128 + nn * 512
                        nc.tensor.matmul(ps[:, :nsz], kT[0][:, bj * 128:(bj + 1) * 128],
                                         qsT[0][:, i0:i0 + nsz], start=True, stop=False)
                        nc.tensor.matmul(ps[:, :nsz], kT[1][:, bj * 128:(bj + 1) * 128],
                                         qsT[1][:, i0:i0 + nsz], start=False, stop=True)
                        # alternate the PSUM evacuation between DVE and ACT so
                        # neither engine serializes the S -> AllReduce path
                        if (bj + nn) % 2 == 0:
                            nc.vector.tensor_copy(sblk[:, nn * 512:nn * 512 + nsz],
                                                  ps[:, :nsz])
                        else:
                            nc.scalar.activation(sblk[:, nn * 512:nn * 512 + nsz],
                                                 ps[:, :nsz], AF.Copy)
                    nc.sync.dma_start(st_w[n][:, col0:col0 + L], sblk[:, :L])
                if NO_AR:
                    nc.gpsimd.dma_start(st_r[n][:], st_w[n][:])
                else:
                    nc.gpsimd.collective_compute(
                        "AllReduce", ALU.add,
                        replica_groups=[[0, 1, 2, 3], [4, 5, 6, 7]],
                        ins=[st_w[n][:]], outs=[st_r[n][:]])

            # v: fp16 GEMM, token-major, 65-wide per-head slots (ones column
            # comes from a zero weight column + the +1 in bva)
            for tb in range(NB):
                ps = vps.tile([128, DV], dt.float32, tag="v_ps")
                for c in range(CC):
                    nc.tensor.matmul(
                        ps[:], x16[:, c * T + tb * 128: c * T + (tb + 1) * 128],
                        wv_t[:, c * DV:(c + 1) * DV],
                        start=(c == 0), stop=(c == CC - 1))
                nc.vector.tensor_add(vaug[:, tb * DV:(tb + 1) * DV], ps[:], bva_t[:])

        # ============ phase 1b + 2, interleaved per i-super ==================
        # PSUM pool order matters: ptp reuses the banks freed earliest
        # (qk/s GEMMs), pjp reuses the v banks (freed last, needed last).
        with tc.tile_pool(name="p2pt", bufs=2, space="PSUM") as ptp, \
             tc.tile_pool(name="p2yt", bufs=2, space="PSUM") as ytp, \
             tc.tile_pool(name="p2pj", bufs=1, space="PSUM") as pjp:
            for s in range(NS):
                # FF^T scan for AR chunk s (blocks 4s..4s+3): i-super s only
                # needs ffst[0..4s+3], so attention for super s starts after
                # chunk s arrives while later chunks are still reducing.
                for bj in range(4 * s, 4 * s + 4):
                    L = BLK_LEN[bj]
                    chunk, col0 = BLK_OFF[bj]
                    sta = p1sta.tile([128, T], S_DT, tag="sta")
                    # gpsimd queue: the collectives ahead of it are async
                    # triggers; on the sync queue this load would sit behind
                    # the previous super's out-writes in SP program order
                    nc.gpsimd.dma_start(sta[:, :L], st_r[chunk][:, col0:col0 + L])
                    nc.vector.tensor_scalar_max(sta[:, :L], sta[:, :L], 0.0)
                    if bj == 0:
                        nc.vector.memset(sta[0:1, :L], 0.0)
                    nc.vector.tensor_mul(sta[:, 0:128], sta[:, 0:128], maskm_t)
                    ff = ffst[bj]
                    nc.vector.memset(ff[:, 0:1], 0.0)
                    if L > 1:
                        # state -= S  => ff holds -cumsum(S) (exclusive)
                        nc.vector.tensor_tensor_scan(
                            ff[:, 1:L], zeros_t[:, 0:L - 1], sta[:, 0:L - 1],
                            0.0, ALU.add, ALU.subtract)
                    # causal/diagonal mask: -60000 where i < j
                    nc.vector.tensor_add(ff[:, 0:128], ff[:, 0:128], maska_t)
                yt_sb = [p2y.tile([128, 512], dt.float16, name="ytA", tag="ytA"),
                         p2y.tile([64, 512], dt.float16, name="ytB", tag="ytB")]
                for h in range(HPC):
                    # head h dims live at rows [h*64, h*64+64) of the m-tiles
                    (qsrc, qof) = (0, h * 64) if h < 2 else (1, 0)
                    yt_ps = ytp.tile([65, 512], dt.float32, tag="yt_ps")
                    # j-blocks in pairs sharing one 2-bank PSUM tile; the FF
                    # subtraction rides the accumulation as ident @ (-FF).
                    # attV matmuls are emitted one pair late so the PE stream
                    # never stalls on the exp of the pair it just produced.
                    pend = None
                    for pj in range(2 * s + 2):
                        pt = ptp.tile([128, 1024], dt.float32, tag="pt")
                        et = p2sb.tile([128, 1024], dt.float16, tag="et")
                        spans = []
                        for half, bj in ((0, 2 * pj), (1, 2 * pj + 1)):
                            delta = bj - 4 * s
                            ioff = 128 * delta if delta >= 0 else 0
                            npr = 512 - ioff
                            i0 = s * 512 + ioff          # global i start
                            floc = i0 - bj * 128         # col offset inside ffst[bj]
                            co = 512 * half
                            nc.tensor.matmul(pt[:, co:co + npr],
                                             kT[qsrc][qof:qof + 64, bj * 128:(bj + 1) * 128],
                                             qT[qsrc][qof:qof + 64, i0:i0 + npr],
                                             start=True, stop=False)
                            nc.tensor.matmul(pt[:, co:co + npr], ident_t,
                                             ffst[bj][:, floc:floc + npr],
                                             start=False, stop=True)
                            spans.append((bj, ioff, npr, co))
                        if spans[0][2] == 512:   # contiguous: one exp
                            w = 512 + spans[1][2]
                            nc.scalar.activation(et[:, :w], pt[:, :w], AF.Exp)
                        else:                    # gap: exp only valid halves
                            for bj, ioff, npr, co in spans:
                                nc.scalar.activation(et[:, co:co + npr],
                                                     pt[:, co:co + npr], AF.Exp)
                        if pend is not None:
                            pspans, pet = pend
                            for bj, ioff, npr, co in pspans:
                                vbase = bj * DV + h * 65
                                nc.tensor.matmul(yt_ps[:, ioff:512],
                                                 vaug[:, vbase:vbase + 65],
                                                 pet[:, co:co + npr],
                                                 start=(bj == 0), stop=False)
                        pend = (spans, et)
                    pspans, pet = pend
                    for bj, ioff, npr, co in pspans:
                        vbase = bj * DV + h * 65
                        nc.tensor.matmul(yt_ps[:, ioff:512],
                                         vaug[:, vbase:vbase + 65],
                                         pet[:, co:co + npr],
                                         start=(bj == 0), stop=(bj == 4 * s + 3))
                    # normalize: yt[d, i] * (1 / sumexp[i])
                    rs = p2sb.tile([1, 512], dt.float16, tag="rs")
                    bc_ps = ptp.tile([64, 512], dt.float32, tag="pt")
                    with nc.allow_low_precision(reason="1/Z fp16: 5e-4 rel vs 2e-2 gate"):
                        nc.vector.reciprocal(rs[:], yt_ps[64:65, :])
                        nc.tensor.matmul(bc_ps[:], c16_t[0:1, C16_ONES:C16_ONES + 64],
                                         rs[:], start=True, stop=True)
                    bc_sb = p2sb.tile([64, 512], dt.float32, tag="bc_sb")
                    nc.vector.tensor_copy(bc_sb[:], bc_ps[:])
                    (dsti, dof) = (0, h * 64) if h < 2 else (1, 0)
                    nc.vector.tensor_mul(yt_sb[dsti][dof:dof + 64, :],
                                         yt_ps[0:64, :], bc_sb[:])
                # output projection for this i-super
                for ib in range(4):
                    po = pjp.tile([128, C], dt.float32, tag="po")
                    for nof, nsz in ((0, 512), (512, 256)):
                        nc.tensor.matmul(po[:, nof:nof + nsz],
                                         yt_sb[0][:, ib * 128:(ib + 1) * 128],
                                         wp_t[0][:, nof:nof + nsz],
                                         start=True, stop=False)
                        nc.tensor.matmul(po[:, nof:nof + nsz],
                                         yt_sb[1][:, ib * 128:(ib + 1) * 128],
                                         wp_t[1][:, nof:nof + nsz],
                                         start=False, stop=True)
                    ost = p2o.tile([128, C], dt.float32, tag="ost")
                    nc.vector.tensor_copy(ost[:], po[:])
                    r0 = s * 512 + ib * 128
                    nc.sync.dma_start(out[r0:r0 + 128, :], ost[:])


def _swizzle(w, width):
    """[CC*128, width] -> [128, CC*width] with chunk c at cols [c*width, ...)."""
    return np.ascontiguousarray(
        w.reshape(CC, 128, width).transpose(1, 0, 2).reshape(128, CC * width))


def _prep_core_inputs(x, w_attn, b_attn, w_proj, b_proj, sel_w, core):
    b, g = core // 4, core % 4
    h0 = 3 * g
    rows = slice(64 * h0, 64 * (h0 + HPC))
    f32, f16 = np.float32, np.float16
    wq = (w_attn[rows, :].T * SCALE).astype(f16)                       # [768, 192]
    wk = w_attn[C + 64 * h0: C + 64 * (h0 + HPC), :].T.astype(f16)
    wv = w_attn[2 * C + 64 * h0: 2 * C + 64 * (h0 + HPC), :].T.astype(f16)
    wv_aug = np.zeros((C, DV), f16)                                    # ones-col slot
    for h in range(HPC):
        wv_aug[:, h * 65: h * 65 + 64] = wv[:, h * 64:(h + 1) * 64]
    wA = np.concatenate([_swizzle(wq, D), _swizzle(wk, D), _swizzle(wv_aug, DV)], axis=1)

    bva = np.zeros((1, DV), f32)
    for h in range(HPC):
        bva[0, h * 65: h * 65 + 64] = b_attn[2 * C + 64 * (h0 + h): 2 * C + 64 * (h0 + h + 1)]
        bva[0, h * 65 + 64] = 1.0
    c16 = np.zeros((128, C16_W), f16)
    c16[:, C16_MASKM:C16_MASKM + 128] = np.triu(np.ones((128, 128), f32), 1)
    c16[:, C16_MASKA:C16_MASKA + 128] = np.tril(np.full((128, 128), NEGBIG, f32), -1)
    c16[:, C16_ID:C16_ID + 128] = np.eye(128, dtype=f32)
    c16[:, C16_ONES:C16_ONES + 64] = 1.0
    c16[:, C16_BVA:C16_BVA + DV] = np.tile(bva, (128, 1))
    c16[:, C16_WPA:C16_WPA + C] = w_proj[:, 64 * h0: 64 * h0 + 128].T.astype(f16)
    c16[0:64, C16_WPB:C16_WPB + C] = w_proj[:, 64 * h0 + 128: 64 * h0 + 192].T.astype(f16)

    c32 = np.zeros((128, 6), f32)
    c32[:, 0] = b_attn[64 * h0: 64 * h0 + 128] * np.float32(SCALE)
    c32[0:64, 1] = b_attn[64 * (h0 + 2): 64 * (h0 + 3)] * np.float32(SCALE)
    c32[:, 2] = b_attn[C + 64 * h0: C + 64 * h0 + 128]
    c32[0:64, 3] = b_attn[C + 64 * (h0 + 2): C + 64 * (h0 + 3)]
    c32[:, 4] = np.repeat(sel_w.astype(f32)[h0:h0 + 2], HD)
    c32[0:64, 5] = np.repeat(sel_w.astype(f32)[h0 + 2:h0 + 3], HD)

    return {
        "xT": _swizzle(np.ascontiguousarray(x[b].T).astype(f16), T),
        "wA": wA,
        "c16": c16,
        "c32": c32,
    }


def kernel(x, w_attn, b_attn, w_proj, b_proj, sel_w):
    x = np.asarray(x); w_attn = np.asarray(w_attn); b_attn = np.asarray(b_attn)
    w_proj = np.asarray(w_proj); b_proj = np.asarray(b_proj); sel_w = np.asarray(sel_w)
    nc = build_nc()
    in_maps = [_prep_core_inputs(x, w_attn, b_attn, w_proj, b_proj, sel_w, c)
               for c in range(N_CORES)]
    res = run_bass_kernel_spmd(nc, in_maps, list(range(N_CORES)))
    out = np.zeros((B, T, C), np.float32)
    for c in range(N_CORES):
        out[c // 4] += res.results[c]["out"]
    out += b_proj.astype(np.float32)
    return out
